# revision 1
# baseline (speedup 1.0000x reference)
"""Trainium2 Bass kernel for MF embedding-lookup + dot-product scoring.

out[u, i] = dot(user_hiddens[user_ids[u]], item_hiddens[item_ids[i]])

Sharding: 2D over 8 cores — 4 user groups (1024 users) x 2 item groups
(2048 items); tables replicated to every core's HBM. Per core:
  - indirect-DMA gathers 128 rows/call (one index per partition), 8 user
    calls + 16 item calls
  - PE transpose to [64, batch]; split each f32 value into bf16 hi+lo
  - per item tile: 3-term bf16 matmuls (hi*hi + hi*lo + lo*hi) accumulate
    in f32 PSUM -> ~1e-5 rel err at ~4x the fp32 matmul speed
  - item tile stationary, users moving: the matmul for item tile t fires
    as soon as tile t's gather lands (no global barrier on the gathers)
  - output [2048 items, 1024 users] written in 512 KB contiguous chunks
Host transposes each core slab into the final [4096, 4096].
"""

import numpy as np

import concourse.bacc as bacc
import concourse.bass as bass
import concourse.mybir as mybir
import concourse.tile as tile
from concourse.bass_utils import run_bass_kernel_spmd
from concourse.masks import make_identity

NUM_USERS = 1_000_000
NUM_ITEMS = 100_000
D = 64
BU = 4096
BI = 4096
N_CORES = 8
RU = 4              # user groups
RI = 2              # item groups
UC = BU // RU       # users per core = 1024
IC = BI // RI       # items per core = 2048
P = 128
UT = UC // P        # user tiles per core = 8
IT = IC // P        # item tiles per core = 16
NBLK = 512          # matmul moving free dim
NH = UC // NBLK     # user halves per item tile = 2

_cache = {}


def _build():
    nc = bacc.Bacc()
    ut_dram = nc.dram_tensor(
        "user_table", [NUM_USERS, D], mybir.dt.float32, kind="ExternalInput"
    )
    it_dram = nc.dram_tensor(
        "item_table", [NUM_ITEMS, D], mybir.dt.float32, kind="ExternalInput"
    )
    uid_dram = nc.dram_tensor("uids", [P, UT], mybir.dt.int32, kind="ExternalInput")
    iid_dram = nc.dram_tensor("iids", [P, IT], mybir.dt.int32, kind="ExternalInput")
    out_dram = nc.dram_tensor(
        "out", [IC, UC], mybir.dt.float32, kind="ExternalOutput"
    )

    f32 = mybir.dt.float32
    bf16 = mybir.dt.bfloat16

    with tile.TileContext(nc) as tc:
        with (
            tc.tile_pool(name="const", bufs=1) as constp,
            tc.tile_pool(name="idx", bufs=1) as idxp,
            tc.tile_pool(name="gath", bufs=24) as gathp,
            tc.tile_pool(name="ops", bufs=1) as opsp,
            tc.tile_pool(name="vt", bufs=4) as vtp,
            tc.tile_pool(name="tp", bufs=2, space="PSUM") as tpp,
            tc.tile_pool(name="mm", bufs=3, space="PSUM") as mmp,
            tc.tile_pool(name="outp", bufs=3) as outp,
        ):
            ident = constp.tile([P, P], f32)
            make_identity(nc, ident[:])

            uids = idxp.tile([P, UT], mybir.dt.int32)
            iids = idxp.tile([P, IT], mybir.dt.int32)
            nc.sync.dma_start(out=uids[:], in_=uid_dram[:])
            nc.sync.dma_start(out=iids[:], in_=iid_dram[:])

            # --- user prologue: gather + transpose + bf16 hi/lo split ---
            # ustack: [uhi; uhi] duplicated across the two partition halves
            # so one K=128 matmul against [vhi; vlo] yields hi*hi + lo_v*hi_u.
            ustack = opsp.tile([2 * D, UC], bf16)
            ulo = opsp.tile([D, UC], bf16)
            for t in range(UT):
                g = gathp.tile([P, D], f32)
                nc.gpsimd.indirect_dma_start(
                    out=g[:],
                    out_offset=None,
                    in_=ut_dram[:],
                    in_offset=bass.IndirectOffsetOnAxis(
                        ap=uids[:, t : t + 1], axis=0
                    ),
                )
                ps = tpp.tile([D, P], f32)
                nc.tensor.transpose(ps[:], g[:], ident[:])
                sl = slice(t * P, (t + 1) * P)
                nc.scalar.copy(out=ustack[0:D, sl], in_=ps[:])
                nc.scalar.copy(out=ustack[D : 2 * D, sl], in_=ps[:])
                nc.vector.tensor_tensor(
                    out=ulo[:, sl],
                    in0=ps[:],
                    in1=ustack[0:D, sl],
                    op=mybir.AluOpType.subtract,
                )

            # --- item stream: gather -> transpose -> hi/lo -> matmuls -> out ---
            for t in range(IT):
                g = gathp.tile([P, D], f32)
                nc.gpsimd.indirect_dma_start(
                    out=g[:],
                    out_offset=None,
                    in_=it_dram[:],
                    in_offset=bass.IndirectOffsetOnAxis(
                        ap=iids[:, t : t + 1], axis=0
                    ),
                )
                ps = tpp.tile([D, P], f32)
                nc.tensor.transpose(ps[:], g[:], ident[:])
                # vstack = [vhi; vlo] on the two partition halves
                vstack = vtp.tile([2 * D, P], bf16)
                nc.scalar.copy(out=vstack[0:D, :], in_=ps[:])
                nc.vector.tensor_tensor(
                    out=vstack[D : 2 * D, :],
                    in0=ps[:],
                    in1=vstack[0:D, :],
                    op=mybir.AluOpType.subtract,
                )

                ot = outp.tile([P, UC], f32)
                po = mmp.tile([P, UC], f32)  # two PSUM banks
                for h in range(NH):
                    hs = slice(h * NBLK, (h + 1) * NBLK)
                    # terms hi_v*hi_u + lo_v*hi_u (K=128 stacked)
                    nc.tensor.matmul(
                        po[:, hs],
                        lhsT=vstack[:, :],
                        rhs=ustack[:, hs],
                        start=True,
                        stop=False,
                    )
                    # term hi_v*lo_u (K=64)
                    nc.tensor.matmul(
                        po[:, hs],
                        lhsT=vstack[0:D, :],
                        rhs=ulo[:, hs],
                        start=False,
                        stop=True,
                    )
                if t % 2 == 0:
                    nc.scalar.copy(out=ot[:], in_=po[:])
                else:
                    nc.vector.tensor_copy(out=ot[:], in_=po[:])
                nc.sync.dma_start(
                    out=out_dram[t * P : (t + 1) * P, :], in_=ot[:]
                )
    nc.finalize()
    return nc


def kernel(user_hiddens, item_hiddens, user_ids, item_ids, **_):
    user_hiddens = np.ascontiguousarray(user_hiddens, dtype=np.float32)
    item_hiddens = np.ascontiguousarray(item_hiddens, dtype=np.float32)
    user_ids = np.asarray(user_ids)
    item_ids = np.asarray(item_ids)

    if "nc" not in _cache:
        _cache["nc"] = _build()
    nc = _cache["nc"]

    in_maps = []
    for c in range(N_CORES):
        cu, ci = divmod(c, RI)
        uc = user_ids[cu * UC : (cu + 1) * UC]
        icd = item_ids[ci * IC : (ci + 1) * IC]
        # [P, T] transposed id layout: idx[p, t] = ids[t*128 + p]
        uids_t = np.ascontiguousarray(uc.astype(np.int32).reshape(UT, P).T)
        iids_t = np.ascontiguousarray(icd.astype(np.int32).reshape(IT, P).T)
        in_maps.append(
            {
                "user_table": user_hiddens,
                "item_table": item_hiddens,
                "uids": uids_t,
                "iids": iids_t,
            }
        )

    res = run_bass_kernel_spmd(nc, in_maps, list(range(N_CORES)))
    out = np.empty((BU, BI), dtype=np.float32)
    for c in range(N_CORES):
        cu, ci = divmod(c, RI)
        out[cu * UC : (cu + 1) * UC, ci * IC : (ci + 1) * IC] = res.results[c][
            "out"
        ].T
    return out



# revision 11
# speedup vs baseline: 1.1534x; 1.1534x over previous
"""Trainium2 Bass kernel for MF embedding-lookup + dot-product scoring.

out[u, i] = dot(user_hiddens[user_ids[u]], item_hiddens[item_ids[i]])

Sharding: 2D over 8 cores - 4 user groups (1024 users) x 2 item groups
(2048 items). Per core:
  - users: 8 indirect-DMA gathers (128 rows each, one index/partition -
    the HW consumes exactly one offset per partition per call)
  - items: sorted on host into 4 equal windows of 512; each window's ids
    fall in a 32768-row span, so a single InstDMAGatherAnt with int16
    local indices fetches 512 rows per call (994ns + 0.34ns/row of Q7
    time instead of ~1us per 128 rows). The table content for each
    window is uploaded per-core as a contiguous [32768, 64] slice - a
    blind range copy; the per-row lookup happens on device.
  - PE warm-up transposes release the HAM clock gate (1.2 -> 2.4 GHz)
    before the real matmuls
  - f32 pair transposes ([128,128] -> two [64,128] tiles); fp16
    materializes during the PSUM->SBUF copies (no separate converts)
  - 32 matmuls: lhsT = uT user tile [64,128] fp16 stationary, rhs = iT
    [64, 512] windows moving, f32 PSUM; copies to fp16 out buffers on
    scalar/vector; 16 output DMAs on sync into [128, 8, 2048] DRAM
Host assembles the full [4096, 4096] f32, unpermuting the sorted item
columns. fp16 keeps rel err ~1e-3, well under the 2e-2 gate.
"""

import numpy as np

import concourse.bacc as bacc
import concourse.bass as bass
import concourse.mybir as mybir
import concourse.tile as tile
from concourse.bass_utils import run_bass_kernel_spmd
from concourse.masks import make_identity

NUM_USERS = 1_000_000
NUM_ITEMS = 100_000
D = 64
BU = 4096
BI = 4096
N_CORES = 8
RU = 4              # user groups
RI = 2              # item groups
UC = BU // RU       # users per core = 1024
IC = BI // RI       # items per core = 2048
P = 128
UT = UC // P        # user tiles per core = 8
WROWS = 32768       # dma_gather window: int16-addressable table rows
NWARM = 10          # PE warm-up transposes (~3.4us to release clock gate)

_cache = {}


def _build(W):
    """W = number of item windows per core (IC/W ids each, int16 range)."""
    IW = IC // W            # ids per window
    ICH = IW // P           # 128-chunks per window
    nc = bacc.Bacc()
    f16 = mybir.dt.float16
    f32 = mybir.dt.float32

    ut_dram = nc.dram_tensor(
        "user_table", [NUM_USERS, D], f32, kind="ExternalInput"
    )
    itabs = [
        nc.dram_tensor(f"itab{w}", [WROWS, D], f32, kind="ExternalInput")
        for w in range(W)
    ]
    uid_dram = nc.dram_tensor("uids", [P, UT], mybir.dt.int32, kind="ExternalInput")
    iidx_dram = nc.dram_tensor(
        "iidx", [P, (IW // 16) * W], mybir.dt.int16, kind="ExternalInput"
    )
    # out[p, ut, i] = score(user ut*128+p, sorted-item i)
    out_dram = nc.dram_tensor("out", [P, UT, IC], f16, kind="ExternalOutput")

    with tile.TileContext(nc) as tc:
        with (
            tc.tile_pool(name="const", bufs=1) as constp,
            tc.tile_pool(name="idx", bufs=1) as idxp,
            tc.tile_pool(name="gath", bufs=1) as gathp,
            tc.tile_pool(name="ops", bufs=1) as opsp,
            tc.tile_pool(name="tp", bufs=2, space="PSUM") as tpp,
            tc.tile_pool(name="mm", bufs=2, space="PSUM") as mmp,
            tc.tile_pool(name="outp", bufs=3) as outp,
        ):
            ident = constp.tile([P, P], f32)
            make_identity(nc, ident[:])

            uids = idxp.tile([P, UT], mybir.dt.int32)
            iidx = idxp.tile([P, (IW // 16) * W], mybir.dt.int16)
            nc.sync.dma_start(out=uids[:], in_=uid_dram[:])
            nc.scalar.dma_start(out=iidx[:], in_=iidx_dram[:])

            # PE warm-up: keep the array busy so the HAM clock gate opens
            for _ in range(NWARM):
                wt = tpp.tile([P, P], f32)
                nc.tensor.transpose(wt[:], ident[:], ident[:])

            # --- gathers on gpsimd (Q7), interleaved for pipelining ---
            igs = [
                gathp.tile([P, ICH, D], f32, name=f"ig{w}") for w in range(W)
            ]
            ug = gathp.tile([P, UT * D], f32)

            def gather_w(w):
                nc.gpsimd.dma_gather(
                    out_ap=igs[w][:],
                    in_ap=itabs[w][:],
                    idxs_ap=iidx[:, w * (IW // 16) : (w + 1) * (IW // 16)],
                    num_idxs=IW,
                    num_idxs_reg=IW,
                    elem_size=D,
                )

            def gather_u(t):
                nc.gpsimd.indirect_dma_start(
                    out=ug[:, t * D : (t + 1) * D],
                    out_offset=None,
                    in_=ut_dram[:],
                    in_offset=bass.IndirectOffsetOnAxis(
                        ap=uids[:, t : t + 1], axis=0
                    ),
                )

            gather_w(0)
            if W > 1:
                gather_w(1)
            gather_u(0)
            gather_u(1)
            for w in range(2, W):
                gather_w(w)
            for t in range(2, UT):
                gather_u(t)

            # --- f32 pair transposes; fp16 lands in the psum->sbuf copy ---
            uT = opsp.tile([D, UC], f16)        # [64, 1024]
            iT = opsp.tile([D, IC], f16)        # [64, 2048]
            cpe = [nc.scalar, nc.vector]

            def _copy(e, out, in_):
                if e is nc.scalar:
                    e.copy(out=out, in_=in_)
                else:
                    e.tensor_copy(out=out, in_=in_)

            def t_item_pair(w, pr):
                ps = tpp.tile([P, P], f32)
                nc.tensor.transpose(
                    ps[:], igs[w][:, 2 * pr : 2 * pr + 2, :], ident[:]
                )
                base = (w * ICH + 2 * pr) * P
                _copy(cpe[pr % 2], iT[:, base : base + P], ps[0:D, :])
                _copy(cpe[(pr + 1) % 2], iT[:, base + P : base + 2 * P], ps[D:P, :])

            def t_user_pair(c):
                ps = tpp.tile([P, P], f32)
                nc.tensor.transpose(ps[:], ug[:, c * P : (c + 1) * P], ident[:])
                base = 2 * c * P
                _copy(cpe[c % 2], uT[:, base : base + P], ps[0:D, :])
                _copy(cpe[(c + 1) % 2], uT[:, base + P : base + 2 * P], ps[D:P, :])

            # --- matmul stream: user tile stationary, item windows moving ---
            NB = 512
            HH = IC // NB           # 4 half-chunks of items
            obs = [None] * UT

            def mm_group(ut_i, hpair):
                po = mmp.tile([P, 2 * NB], f32)
                for k in range(2):
                    h = 2 * hpair + k
                    nc.tensor.matmul(
                        po[:, k * NB : (k + 1) * NB],
                        lhsT=uT[:, ut_i * P : (ut_i + 1) * P],
                        rhs=iT[:, h * NB : (h + 1) * NB],
                    )
                if obs[ut_i] is None:
                    obs[ut_i] = outp.tile([P, IC], f16, name=f"ob{ut_i}")
                ob = obs[ut_i]
                sl = slice(hpair * 2 * NB, (hpair + 1) * 2 * NB)
                _copy(cpe[(ut_i + hpair) % 2], ob[:, sl], po[:])
                nc.sync.dma_start(
                    out=out_dram[:, ut_i : ut_i + 1, sl], in_=ob[:, sl]
                )

            if W == 4:
                # windows 0,1 transposed -> first MM half-pair for early tiles
                t_item_pair(0, 0)
                t_item_pair(0, 1)
                t_item_pair(1, 0)
                t_item_pair(1, 1)
                t_user_pair(0)
                mm_group(0, 0)
                mm_group(1, 0)
                t_item_pair(2, 0)
                t_item_pair(2, 1)
                t_item_pair(3, 0)
                t_item_pair(3, 1)
                mm_group(0, 1)
                mm_group(1, 1)
                for c in range(1, UT // 2):
                    t_user_pair(c)
                    mm_group(2 * c, 0)
                    mm_group(2 * c, 1)
                    mm_group(2 * c + 1, 0)
                    mm_group(2 * c + 1, 1)
            else:
                for w in range(W):
                    for pr in range(ICH // 2):
                        t_item_pair(w, pr)
                for c in range(UT // 2):
                    t_user_pair(c)
                for ut_i in range(UT):
                    for hpair in range(HH // 2):
                        mm_group(ut_i, hpair)
    nc.finalize()
    return nc


def kernel(user_hiddens, item_hiddens, user_ids, item_ids, **_):
    user_ids = np.asarray(user_ids)
    item_ids = np.asarray(item_ids)
    utab = np.ascontiguousarray(np.asarray(user_hiddens), dtype=np.float32)
    itab = np.ascontiguousarray(np.asarray(item_hiddens), dtype=np.float32)

    # per-item-group window split (sorted; W windows of IC/W ids, each
    # spanning < 32768 table rows so local indices fit int16)
    groups = []
    W = 4
    for ci in range(RI):
        ids = item_ids[ci * IC : (ci + 1) * IC].astype(np.int64)
        perm = np.argsort(ids, kind="stable")
        srt = ids[perm]
        while W <= 16:
            IW = IC // W
            spans = [
                srt[(w + 1) * IW - 1] - srt[w * IW] for w in range(W)
            ]
            if all(s < WROWS for s in spans):
                break
            W *= 2
        groups.append((perm, srt))
    IW = IC // W

    if ("nc", W) not in _cache:
        _cache[("nc", W)] = _build(W)
    nc = _cache[("nc", W)]

    gdata = []
    for ci in range(RI):
        perm, srt = groups[ci]
        slices = []
        iidx16 = np.empty((16, (IW // 16) * W), dtype=np.int16)
        for w in range(W):
            b = int(srt[w * IW])
            sl = itab[b : b + WROWS]
            if sl.shape[0] < WROWS:
                sl = np.pad(sl, ((0, WROWS - sl.shape[0]), (0, 0)))
            slices.append(np.ascontiguousarray(sl))
            loc = (srt[w * IW : (w + 1) * IW] - b).astype(np.int16)
            iidx16[:, w * (IW // 16) : (w + 1) * (IW // 16)] = loc.reshape(
                IW // 16, 16
            ).T
        iidx_full = np.ascontiguousarray(np.tile(iidx16, (8, 1)))
        gdata.append((perm, slices, iidx_full))

    in_maps = []
    for c in range(N_CORES):
        cu, ci = divmod(c, RI)
        uc = user_ids[cu * UC : (cu + 1) * UC]
        uids_t = np.ascontiguousarray(uc.astype(np.int32).reshape(UT, P).T)
        perm, slices, iidx_full = gdata[ci]
        m = {"user_table": utab, "uids": uids_t, "iidx": iidx_full}
        for w in range(W):
            m[f"itab{w}"] = slices[w]
        in_maps.append(m)

    res = run_bass_kernel_spmd(nc, in_maps, list(range(N_CORES)))
    out = np.empty((BU, BI), dtype=np.float32)
    for c in range(N_CORES):
        cu, ci = divmod(c, RI)
        perm = gdata[ci][0]
        # res "out" [128, 8, 2048]: [p, ut, sorted-item] -> [ut*128+p, item]
        slab = res.results[c]["out"].transpose(1, 0, 2).reshape(UC, IC)
        blk = np.empty((UC, IC), dtype=np.float32)
        blk[:, perm] = slab.astype(np.float32)
        out[cu * UC : (cu + 1) * UC, ci * IC : (ci + 1) * IC] = blk
    return out


# revision 12
# speedup vs baseline: 1.1726x; 1.0167x over previous
"""Trainium2 Bass kernel for MF embedding-lookup + dot-product scoring.

out[u, i] = dot(user_hiddens[user_ids[u]], item_hiddens[item_ids[i]])

Sharding: 2D over 8 cores - 4 user groups (1024 users) x 2 item groups
(2048 items); tables replicated in every core's HBM. Per core:
  - 24 indirect-DMA gathers (128 rows / one index per partition each),
    items first then users, so downstream compute chases the Q7
    descriptor-generation stream
  - PE warm-up transposes release the HAM clock gate (1.2 -> 2.4 GHz)
    before the real work
  - f32 pair transposes ([128,128] -> two [64,128] tiles); fp16
    materializes during the PSUM->SBUF copies (no separate converts)
  - 32 matmuls: lhsT = uT user tile [64,128] fp16 stationary, rhs = iT
    [64, 512] item chunks moving, f32 PSUM accumulate
  - PSUM->SBUF fp16 copies alternate scalar/vector; 16 output DMAs on
    sync into a [128, 8, 2048] DRAM layout (2KB/partition descriptors)
Host assembles the full [4096, 4096] f32 output from the fp16 slabs.
fp16 keeps rel err ~5e-4, well under the 2e-2 gate.
"""

import numpy as np

import concourse.bacc as bacc
import concourse.bass as bass
import concourse.mybir as mybir
import concourse.tile as tile
from concourse.bass_utils import run_bass_kernel_spmd
from concourse.masks import make_identity

NUM_USERS = 1_000_000
NUM_ITEMS = 100_000
D = 64
BU = 4096
BI = 4096
N_CORES = 8
RU = 4              # user groups
RI = 2              # item groups
UC = BU // RU       # users per core = 1024
IC = BI // RI       # items per core = 2048
P = 128
UT = UC // P        # user tiles per core = 8
IT = IC // P        # item tiles per core = 16
NWARM = 10          # PE warm-up transposes (~3.4us to release clock gate)

_cache = {}


def _build():
    nc = bacc.Bacc()
    f16 = mybir.dt.float16
    f32 = mybir.dt.float32

    ut_dram = nc.dram_tensor(
        "user_table", [NUM_USERS, D], f32, kind="ExternalInput"
    )
    it_dram = nc.dram_tensor(
        "item_table", [NUM_ITEMS, D], f32, kind="ExternalInput"
    )
    uid_dram = nc.dram_tensor("uids", [P, UT], mybir.dt.int32, kind="ExternalInput")
    iid_dram = nc.dram_tensor("iids", [P, IT], mybir.dt.int32, kind="ExternalInput")
    # out[p, ut, i] = score(user ut*128+p, item i)
    out_dram = nc.dram_tensor("out", [P, UT, IC], f16, kind="ExternalOutput")

    with tile.TileContext(nc) as tc:
        with (
            tc.tile_pool(name="const", bufs=1) as constp,
            tc.tile_pool(name="idx", bufs=1) as idxp,
            tc.tile_pool(name="gath", bufs=1) as gathp,
            tc.tile_pool(name="ops", bufs=1) as opsp,
            tc.tile_pool(name="tp", bufs=2, space="PSUM") as tpp,
            tc.tile_pool(name="mm", bufs=2, space="PSUM") as mmp,
            tc.tile_pool(name="outp", bufs=3) as outp,
        ):
            ident = constp.tile([P, P], f32)
            make_identity(nc, ident[:])

            uids = idxp.tile([P, UT], mybir.dt.int32)
            iids = idxp.tile([P, IT], mybir.dt.int32)
            nc.sync.dma_start(out=uids[:], in_=uid_dram[:])
            nc.scalar.dma_start(out=iids[:], in_=iid_dram[:])

            # PE warm-up: keep the array busy so the HAM clock gate opens
            for _ in range(NWARM):
                wt = tpp.tile([P, P], f32)
                nc.tensor.transpose(wt[:], ident[:], ident[:])

            ig = gathp.tile([P, IT * D], f32)
            ug = gathp.tile([P, UT * D], f32)

            def gather(out_sl, table, ids_col):
                nc.gpsimd.indirect_dma_start(
                    out=out_sl,
                    out_offset=None,
                    in_=table[:],
                    in_offset=bass.IndirectOffsetOnAxis(ap=ids_col, axis=0),
                )

            # items first (they gate every matmul's rhs chunks), users chase
            for t in range(4):
                gather(ig[:, t * D : (t + 1) * D], it_dram, iids[:, t : t + 1])
            gather(ug[:, 0:D], ut_dram, uids[:, 0:1])
            gather(ug[:, D : 2 * D], ut_dram, uids[:, 1:2])
            for t in range(4, IT):
                gather(ig[:, t * D : (t + 1) * D], it_dram, iids[:, t : t + 1])
            for t in range(2, UT):
                gather(ug[:, t * D : (t + 1) * D], ut_dram, uids[:, t : t + 1])

            uT = opsp.tile([D, UC], f16)        # [64, 1024]
            iT = opsp.tile([D, IC], f16)        # [64, 2048]
            cpe = [nc.scalar, nc.vector]

            def _copy(e, out, in_):
                if e is nc.scalar:
                    e.copy(out=out, in_=in_)
                else:
                    e.tensor_copy(out=out, in_=in_)

            def t_pair(src, dst, c):
                ps = tpp.tile([P, P], f32)
                nc.tensor.transpose(ps[:], src[:, c * P : (c + 1) * P], ident[:])
                base = 2 * c * P
                _copy(cpe[c % 2], dst[:, base : base + P], ps[0:D, :])
                _copy(cpe[(c + 1) % 2], dst[:, base + P : base + 2 * P], ps[D:P, :])

            NB = 512
            obs = [None] * UT

            def mm_group(ut_i, hpair):
                po = mmp.tile([P, 2 * NB], f32)
                for k in range(2):
                    h = 2 * hpair + k
                    nc.tensor.matmul(
                        po[:, k * NB : (k + 1) * NB],
                        lhsT=uT[:, ut_i * P : (ut_i + 1) * P],
                        rhs=iT[:, h * NB : (h + 1) * NB],
                    )
                if obs[ut_i] is None:
                    obs[ut_i] = outp.tile([P, IC], f16, name=f"ob{ut_i}")
                ob = obs[ut_i]
                sl = slice(hpair * 2 * NB, (hpair + 1) * 2 * NB)
                _copy(cpe[(ut_i + hpair) % 2], ob[:, sl], po[:])
                nc.sync.dma_start(
                    out=out_dram[:, ut_i : ut_i + 1, sl], in_=ob[:, sl]
                )

            # item pairs 0-3 (tiles 0-7 = rhs chunks h0,h1), user pair 0,
            # then matmuls chase the remaining gathers
            t_pair(ig, iT, 0)
            t_pair(ig, iT, 1)
            t_pair(ig, iT, 2)
            t_pair(ig, iT, 3)
            t_pair(ug, uT, 0)
            mm_group(0, 0)
            mm_group(1, 0)
            t_pair(ig, iT, 4)
            t_pair(ig, iT, 5)
            t_pair(ig, iT, 6)
            t_pair(ig, iT, 7)
            mm_group(0, 1)
            mm_group(1, 1)
            for c in range(1, UT // 2):
                t_pair(ug, uT, c)
                mm_group(2 * c, 0)
                mm_group(2 * c, 1)
                mm_group(2 * c + 1, 0)
                mm_group(2 * c + 1, 1)
    nc.finalize()
    return nc


def kernel(user_hiddens, item_hiddens, user_ids, item_ids, **_):
    user_ids = np.asarray(user_ids)
    item_ids = np.asarray(item_ids)
    utab = np.ascontiguousarray(np.asarray(user_hiddens), dtype=np.float32)
    itab = np.ascontiguousarray(np.asarray(item_hiddens), dtype=np.float32)

    if "nc" not in _cache:
        _cache["nc"] = _build()
    nc = _cache["nc"]

    in_maps = []
    for c in range(N_CORES):
        cu, ci = divmod(c, RI)
        uc = user_ids[cu * UC : (cu + 1) * UC]
        icd = item_ids[ci * IC : (ci + 1) * IC]
        uids_t = np.ascontiguousarray(uc.astype(np.int32).reshape(UT, P).T)
        iids_t = np.ascontiguousarray(icd.astype(np.int32).reshape(IT, P).T)
        in_maps.append(
            {
                "user_table": utab,
                "item_table": itab,
                "uids": uids_t,
                "iids": iids_t,
            }
        )

    res = run_bass_kernel_spmd(nc, in_maps, list(range(N_CORES)))
    out = np.empty((BU, BI), dtype=np.float32)
    for c in range(N_CORES):
        cu, ci = divmod(c, RI)
        # res "out" [128, 8, 2048]: [p, ut, i] -> [ut*128+p, i]
        slab = res.results[c]["out"].transpose(1, 0, 2).reshape(UC, IC)
        out[cu * UC : (cu + 1) * UC, ci * IC : (ci + 1) * IC] = slab.astype(
            np.float32
        )
    return out


# revision 16
# speedup vs baseline: 1.1850x; 1.0105x over previous
"""Trainium2 Bass kernel for MF embedding-lookup + dot-product scoring.

out[u, i] = dot(user_hiddens[user_ids[u]], item_hiddens[item_ids[i]])

Sharding: 2D over 8 cores - 4 user groups (1024 users) x 2 item groups
(2048 items); tables replicated in every core's HBM. Per core:
  - 24 indirect-DMA gathers (128 rows / one index per partition each),
    items first then users, so downstream compute chases the Q7
    descriptor-generation stream
  - PE warm-up transposes release the HAM clock gate (1.2 -> 2.4 GHz)
    before the real work
  - f32 pair transposes ([128,128] -> two [64,128] tiles); fp16
    materializes during the PSUM->SBUF copies (no separate converts)
  - 32 matmuls: lhsT = uT user tile [64,128] fp16 stationary, rhs = iT
    [64, 512] item chunks moving, f32 PSUM accumulate
  - PSUM->SBUF fp16 copies alternate scalar/vector; 16 output DMAs on
    sync into a [128, 8, 2048] DRAM layout (2KB/partition descriptors)
Host assembles the full [4096, 4096] f32 output from the fp16 slabs.
fp16 keeps rel err ~5e-4, well under the 2e-2 gate.
"""

import numpy as np

import concourse.bacc as bacc
import concourse.bass as bass
import concourse.mybir as mybir
import concourse.tile as tile
from concourse.bass_utils import run_bass_kernel_spmd
from concourse.masks import make_identity

NUM_USERS = 1_000_000
NUM_ITEMS = 100_000
D = 64
BU = 4096
BI = 4096
N_CORES = 8
RU = 4              # user groups
RI = 2              # item groups
UC = BU // RU       # users per core = 1024
IC = BI // RI       # items per core = 2048
P = 128
UT = UC // P        # user tiles per core = 8
IT = IC // P        # item tiles per core = 16
NWARM = 10          # PE warm-up transposes (~3.4us to release clock gate)

_cache = {}


def _build():
    nc = bacc.Bacc()
    f16 = mybir.dt.float16
    f32 = mybir.dt.float32

    ut_dram = nc.dram_tensor(
        "user_table", [NUM_USERS, D], f32, kind="ExternalInput"
    )
    it_dram = nc.dram_tensor(
        "item_table", [NUM_ITEMS, D], f32, kind="ExternalInput"
    )
    uid_dram = nc.dram_tensor("uids", [P, UT], mybir.dt.int32, kind="ExternalInput")
    iid_dram = nc.dram_tensor("iids", [P, IT], mybir.dt.int32, kind="ExternalInput")
    # out[p, ut, i] = score(user ut*128+p, item i)
    out_dram = nc.dram_tensor("out", [P, UT, IC], f16, kind="ExternalOutput")

    with tile.TileContext(nc) as tc:
        with (
            tc.tile_pool(name="const", bufs=1) as constp,
            tc.tile_pool(name="idx", bufs=1) as idxp,
            tc.tile_pool(name="gath", bufs=1) as gathp,
            tc.tile_pool(name="ops", bufs=1) as opsp,
            tc.tile_pool(name="tp", bufs=2, space="PSUM") as tpp,
            tc.tile_pool(name="mm", bufs=2, space="PSUM") as mmp,
            tc.tile_pool(name="outp", bufs=3) as outp,
        ):
            ident = constp.tile([P, P], f32)
            make_identity(nc, ident[:])

            uids = idxp.tile([P, UT], mybir.dt.int32)
            iids = idxp.tile([P, IT], mybir.dt.int32)
            nc.sync.dma_start(out=uids[:], in_=uid_dram[:], single_packet=True)
            nc.scalar.dma_start(out=iids[:], in_=iid_dram[:], single_packet=True)

            # PE warm-up: keep the array busy so the HAM clock gate opens
            for _ in range(NWARM):
                wt = tpp.tile([P, P], f32)
                nc.tensor.transpose(wt[:], ident[:], ident[:])

            ig = gathp.tile([P, IT * D], f32)
            ug = gathp.tile([P, UT * D], f32)

            def gather(out_sl, table, ids_col):
                nc.gpsimd.indirect_dma_start(
                    out=out_sl,
                    out_offset=None,
                    in_=table[:],
                    in_offset=bass.IndirectOffsetOnAxis(ap=ids_col, axis=0),
                )

            # order so matmul work arrives steadily and only user pair 3
            # gates the pipeline tail: items 0-7, users 0-3, items 8-15,
            # users 4-7
            for t in range(8):
                gather(ig[:, t * D : (t + 1) * D], it_dram, iids[:, t : t + 1])
            for t in range(4):
                gather(ug[:, t * D : (t + 1) * D], ut_dram, uids[:, t : t + 1])
            for t in range(8, IT):
                gather(ig[:, t * D : (t + 1) * D], it_dram, iids[:, t : t + 1])
            for t in range(4, UT):
                gather(ug[:, t * D : (t + 1) * D], ut_dram, uids[:, t : t + 1])

            uT = opsp.tile([D, UC], f16)        # [64, 1024]
            iT = opsp.tile([D, IC], f16)        # [64, 2048]
            cpe = [nc.scalar, nc.vector]

            def _copy(e, out, in_):
                if e is nc.scalar:
                    e.copy(out=out, in_=in_)
                else:
                    e.tensor_copy(out=out, in_=in_)

            def t_pair(src, dst, c):
                ps = tpp.tile([P, P], f32)
                nc.tensor.transpose(ps[:], src[:, c * P : (c + 1) * P], ident[:])
                base = 2 * c * P
                _copy(cpe[c % 2], dst[:, base : base + P], ps[0:D, :])
                _copy(cpe[(c + 1) % 2], dst[:, base + P : base + 2 * P], ps[D:P, :])

            NB = 512
            obs = [None] * UT

            def mm_group(ut_i, hpair):
                po = mmp.tile([P, 2 * NB], f32)
                for k in range(2):
                    h = 2 * hpair + k
                    nc.tensor.matmul(
                        po[:, k * NB : (k + 1) * NB],
                        lhsT=uT[:, ut_i * P : (ut_i + 1) * P],
                        rhs=iT[:, h * NB : (h + 1) * NB],
                    )
                if obs[ut_i] is None:
                    obs[ut_i] = outp.tile([P, IC], f16, name=f"ob{ut_i}")
                ob = obs[ut_i]
                sl = slice(hpair * 2 * NB, (hpair + 1) * 2 * NB)
                _copy(cpe[(ut_i + hpair) % 2], ob[:, sl], po[:])
                nc.sync.dma_start(
                    out=out_dram[:, ut_i : ut_i + 1, sl], in_=ob[:, sl]
                )

            def warm(n):
                # dependency-free PE filler: holds the HAM clock at 2.4GHz
                # while the PE would otherwise idle waiting on gathers
                for _ in range(n):
                    wt = tpp.tile([P, P], f32, name="wt")
                    nc.tensor.transpose(wt[:], ident[:], ident[:])

            # PE program chases the gather stream; warm() fills idle gaps
            t_pair(ig, iT, 0)
            warm(3)
            t_pair(ig, iT, 1)
            warm(3)
            t_pair(ig, iT, 2)
            warm(3)
            t_pair(ig, iT, 3)
            warm(3)
            t_pair(ug, uT, 0)
            mm_group(0, 0)
            mm_group(1, 0)
            t_pair(ug, uT, 1)
            mm_group(2, 0)
            mm_group(3, 0)
            warm(6)
            t_pair(ig, iT, 4)
            t_pair(ig, iT, 5)
            warm(2)
            t_pair(ig, iT, 6)
            t_pair(ig, iT, 7)
            mm_group(0, 1)
            mm_group(1, 1)
            mm_group(2, 1)
            mm_group(3, 1)
            t_pair(ug, uT, 2)
            mm_group(4, 0)
            mm_group(4, 1)
            mm_group(5, 0)
            mm_group(5, 1)
            t_pair(ug, uT, 3)
            mm_group(6, 0)
            mm_group(6, 1)
            mm_group(7, 0)
            mm_group(7, 1)
    nc.finalize()
    return nc


def kernel(user_hiddens, item_hiddens, user_ids, item_ids, **_):
    user_ids = np.asarray(user_ids)
    item_ids = np.asarray(item_ids)
    utab = np.ascontiguousarray(np.asarray(user_hiddens), dtype=np.float32)
    itab = np.ascontiguousarray(np.asarray(item_hiddens), dtype=np.float32)

    if "nc" not in _cache:
        _cache["nc"] = _build()
    nc = _cache["nc"]

    in_maps = []
    for c in range(N_CORES):
        cu, ci = divmod(c, RI)
        uc = user_ids[cu * UC : (cu + 1) * UC]
        icd = item_ids[ci * IC : (ci + 1) * IC]
        uids_t = np.ascontiguousarray(uc.astype(np.int32).reshape(UT, P).T)
        iids_t = np.ascontiguousarray(icd.astype(np.int32).reshape(IT, P).T)
        in_maps.append(
            {
                "user_table": utab,
                "item_table": itab,
                "uids": uids_t,
                "iids": iids_t,
            }
        )

    res = run_bass_kernel_spmd(nc, in_maps, list(range(N_CORES)))
    out = np.empty((BU, BI), dtype=np.float32)
    for c in range(N_CORES):
        cu, ci = divmod(c, RI)
        # res "out" [128, 8, 2048]: [p, ut, i] -> [ut*128+p, i]
        slab = res.results[c]["out"].transpose(1, 0, 2).reshape(UC, IC)
        out[cu * UC : (cu + 1) * UC, ci * IC : (ci + 1) * IC] = slab.astype(
            np.float32
        )
    return out


# revision 20
# speedup vs baseline: 1.3026x; 1.0993x over previous
"""Trainium2 Bass kernel for MF embedding-lookup + dot-product scoring.

out[u, i] = dot(user_hiddens[user_ids[u]], item_hiddens[item_ids[i]])

Sharding: 2D over 8 cores - 4 user groups (1024 users) x 2 item groups
(2048 items); tables replicated in every core's HBM. Per core:
  - 24 indirect-DMA gathers (128 rows / one index per partition each),
    items first then users, so downstream compute chases the Q7
    descriptor-generation stream
  - PE warm-up transposes release the HAM clock gate (1.2 -> 2.4 GHz)
    before the real work
  - f32 pair transposes ([128,128] -> two [64,128] tiles); fp16
    materializes during the PSUM->SBUF copies (no separate converts)
  - 32 matmuls: lhsT = uT user tile [64,128] fp16 stationary, rhs = iT
    [64, 512] item chunks moving, f32 PSUM accumulate
  - PSUM->SBUF fp16 copies alternate scalar/vector; 16 output DMAs on
    sync into a [128, 8, 2048] DRAM layout (2KB/partition descriptors)
Host assembles the full [4096, 4096] f32 output from the fp16 slabs.
fp16 keeps rel err ~5e-4, well under the 2e-2 gate.
"""

import numpy as np

import concourse.bacc as bacc
import concourse.bass as bass
import concourse.mybir as mybir
import concourse.tile as tile
from concourse.bass_utils import run_bass_kernel_spmd
from concourse.masks import make_identity

NUM_USERS = 1_000_000
NUM_ITEMS = 100_000
D = 64
BU = 4096
BI = 4096
N_CORES = 8
RU = 4              # user groups
RI = 2              # item groups
UC = BU // RU       # users per core = 1024
IC = BI // RI       # items per core = 2048
P = 128
UT = UC // P        # user tiles per core = 8
IT = IC // P        # item tiles per core = 16
NWARM = 10          # PE warm-up transposes (~3.4us to release clock gate)

_cache = {}


def _build():
    nc = bacc.Bacc()
    f16 = mybir.dt.float16
    f32 = mybir.dt.float32

    ut_dram = nc.dram_tensor(
        "user_table", [NUM_USERS, D], f32, kind="ExternalInput"
    )
    it_dram = nc.dram_tensor(
        "item_table", [NUM_ITEMS, D], f32, kind="ExternalInput"
    )
    uid_dram = nc.dram_tensor("uids", [P, UT], mybir.dt.int32, kind="ExternalInput")
    iid_dram = nc.dram_tensor("iids", [P, IT], mybir.dt.int32, kind="ExternalInput")
    # out[p, ut, i] = score(user ut*128+p, item i)
    out_dram = nc.dram_tensor("out", [P, UT, IC], f16, kind="ExternalOutput")

    with tile.TileContext(nc) as tc:
        with (
            tc.tile_pool(name="const", bufs=1) as constp,
            tc.tile_pool(name="idx", bufs=1) as idxp,
            tc.tile_pool(name="gath", bufs=1) as gathp,
            tc.tile_pool(name="ops", bufs=1) as opsp,
            tc.tile_pool(name="tp", bufs=2, space="PSUM") as tpp,
            tc.tile_pool(name="mm", bufs=4, space="PSUM") as mmp,
            tc.tile_pool(name="outp", bufs=3) as outp,
        ):
            ident = constp.tile([P, P], f32)
            make_identity(nc, ident[:])

            uids = idxp.tile([P, UT], mybir.dt.int32)
            iids = idxp.tile([P, IT], mybir.dt.int32)
            nc.sync.dma_start(out=uids[:], in_=uid_dram[:], single_packet=True)
            nc.scalar.dma_start(out=iids[:], in_=iid_dram[:], single_packet=True)

            # PE warm-up: keep the array busy so the HAM clock gate opens
            for _ in range(NWARM):
                wt = tpp.tile([P, P], f32)
                nc.tensor.transpose(wt[:], ident[:], ident[:])

            ig = gathp.tile([P, IT * D], f32)
            ug = gathp.tile([P, UT * D], f32)

            def gather(out_sl, table, ids_col):
                nc.gpsimd.indirect_dma_start(
                    out=out_sl,
                    out_offset=None,
                    in_=table[:],
                    in_offset=bass.IndirectOffsetOnAxis(ap=ids_col, axis=0),
                )

            # order so matmul work arrives steadily and only user pair 3
            # gates the pipeline tail: items 0-7, users 0-5, items 8-15,
            # users 6-7
            for t in range(8):
                gather(ig[:, t * D : (t + 1) * D], it_dram, iids[:, t : t + 1])
            for t in range(6):
                gather(ug[:, t * D : (t + 1) * D], ut_dram, uids[:, t : t + 1])
            for t in range(8, IT):
                gather(ig[:, t * D : (t + 1) * D], it_dram, iids[:, t : t + 1])
            for t in range(6, UT):
                gather(ug[:, t * D : (t + 1) * D], ut_dram, uids[:, t : t + 1])

            uT = opsp.tile([D, UC], f16)        # [64, 1024]
            iT = opsp.tile([D, IC], f16)        # [64, 2048]
            cpe = [nc.scalar, nc.vector]

            def _copy(e, out, in_):
                if e is nc.scalar:
                    e.copy(out=out, in_=in_)
                else:
                    e.tensor_copy(out=out, in_=in_)

            def t_pair(src, dst, c):
                ps = tpp.tile([P, P], f32)
                nc.tensor.transpose(ps[:], src[:, c * P : (c + 1) * P], ident[:])
                base = 2 * c * P
                _copy(cpe[c % 2], dst[:, base : base + P], ps[0:D, :])
                _copy(cpe[(c + 1) % 2], dst[:, base + P : base + 2 * P], ps[D:P, :])

            NB = 512
            obs = [None] * UT

            def mm_group(ut_i, hpair):
                if obs[ut_i] is None:
                    obs[ut_i] = outp.tile([P, IC], f16, name=f"ob{ut_i}")
                ob = obs[ut_i]
                for k in range(2):
                    h = 2 * hpair + k
                    po = mmp.tile([P, NB], f32, name="po")
                    nc.tensor.matmul(
                        po[:],
                        lhsT=uT[:, ut_i * P : (ut_i + 1) * P],
                        rhs=iT[:, h * NB : (h + 1) * NB],
                    )
                    _copy(cpe[(ut_i + h) % 2], ob[:, h * NB : (h + 1) * NB], po[:])
                sl = slice(hpair * 2 * NB, (hpair + 1) * 2 * NB)
                nc.sync.dma_start(
                    out=out_dram[:, ut_i : ut_i + 1, sl], in_=ob[:, sl]
                )

            def warm(n):
                # dependency-free PE filler: holds the HAM clock at 2.4GHz
                # while the PE would otherwise idle waiting on gathers
                for _ in range(n):
                    wt = tpp.tile([P, P], f32, name="wt")
                    nc.tensor.transpose(wt[:], ident[:], ident[:])

            # PE program chases the gather stream; warm() fills idle gaps
            t_pair(ig, iT, 0)
            warm(3)
            t_pair(ig, iT, 1)
            warm(3)
            t_pair(ig, iT, 2)
            warm(3)
            t_pair(ig, iT, 3)
            warm(3)
            t_pair(ug, uT, 0)
            mm_group(0, 0)
            mm_group(1, 0)
            t_pair(ug, uT, 1)
            mm_group(2, 0)
            mm_group(3, 0)
            t_pair(ug, uT, 2)
            mm_group(4, 0)
            mm_group(5, 0)
            warm(4)
            t_pair(ig, iT, 4)
            t_pair(ig, iT, 5)
            t_pair(ig, iT, 6)
            t_pair(ig, iT, 7)
            mm_group(0, 1)
            mm_group(1, 1)
            mm_group(2, 1)
            mm_group(3, 1)
            mm_group(4, 1)
            mm_group(5, 1)
            t_pair(ug, uT, 3)
            mm_group(6, 0)
            mm_group(6, 1)
            mm_group(7, 0)
            mm_group(7, 1)
    nc.finalize()
    return nc


def kernel(user_hiddens, item_hiddens, user_ids, item_ids, **_):
    user_ids = np.asarray(user_ids)
    item_ids = np.asarray(item_ids)
    utab = np.ascontiguousarray(np.asarray(user_hiddens), dtype=np.float32)
    itab = np.ascontiguousarray(np.asarray(item_hiddens), dtype=np.float32)

    if "nc" not in _cache:
        _cache["nc"] = _build()
    nc = _cache["nc"]

    in_maps = []
    for c in range(N_CORES):
        cu, ci = divmod(c, RI)
        uc = user_ids[cu * UC : (cu + 1) * UC]
        icd = item_ids[ci * IC : (ci + 1) * IC]
        uids_t = np.ascontiguousarray(uc.astype(np.int32).reshape(UT, P).T)
        iids_t = np.ascontiguousarray(icd.astype(np.int32).reshape(IT, P).T)
        in_maps.append(
            {
                "user_table": utab,
                "item_table": itab,
                "uids": uids_t,
                "iids": iids_t,
            }
        )

    res = run_bass_kernel_spmd(nc, in_maps, list(range(N_CORES)))
    out = np.empty((BU, BI), dtype=np.float32)
    for c in range(N_CORES):
        cu, ci = divmod(c, RI)
        # res "out" [128, 8, 2048]: [p, ut, i] -> [ut*128+p, i]
        slab = res.results[c]["out"].transpose(1, 0, 2).reshape(UC, IC)
        out[cu * UC : (cu + 1) * UC, ci * IC : (ci + 1) * IC] = slab.astype(
            np.float32
        )
    return out
